# revision 35
# baseline (speedup 1.0000x reference)
"""Trainium2 Bass kernel for nn_ContrastiveLoss (B=512, ZI=16, T=8, D=128).

Strategy: data-parallel over img batch (64 bi per core), text replicated.

v6 design notes:
  - both inputs L2-normalized, d-major transposed, bf16-cast on the host and
    shipped as ONE fused DRAM buffer; two input DMAs (the first covers img +
    the first 4 text blocks) so the q-loop starts ~3us in.
  - per-core q-block permutation of text puts each core's own 4 diagonal
    q-tiles at positions 0-3 (always DVE-routed, so their raw sims come out
    and the host reads the diag contribution directly).
  - the loop works on PAIRS of q-tiles sharing one 4-bank PSUM tile
    [128,2048] (2 pair-bufs = all 8 banks).  PSUM evacuation is split
    across all three eligible paths so DVE, ACT and GpSimd all stream in
    parallel:
      'd'  pair: one DVE reduce_max (1x PSUM, ~2.26us/pair) -> raw sims
      'ad' pair: one ACT exp [128,2048] (~2.0us/pair) -> bf16, then a
                 pair-grouped max-tree on DVE (2x-mode TT, ~1.24us/pair)
      'gp' pair: one ACT exp(32*s) -> bf16, then a pair-grouped ADD-tree on
                 GpSimd (Pool has no MAX ALU, but a sharpened sum
                 (sum_i e^{32 s_i})^{1/32} ~= max_i e^{s_i}; the host takes
                 the 1/32 power).  TT/reduce never grab DVE's shared port,
                 so Pool never blocks DVE.
  - output is just the [128, 32, 64] column blocks (sim for 'd', e for
    'ad', sharpened sums for 'gp'), DMA'd out in 4 chunks during the loop;
    the host finishes the den/diag log-reductions in numpy (f64).
"""
import os
import numpy as np
import ml_dtypes

B, ZI, T, D = 512, 16, 8, 128
NC = 8
BL = B // NC            # 64 local bi
MLOC = BL * ZI          # 1024 img rows per core
NT = B * T              # 4096 text rows
PT = NT // 128          # 32 text partition-tiles (q)
NP = PT // 2            # 16 position pairs
DIAG_COEF = -(1.0 + 1.0 / T)
SHARP = 32.0            # gp-route sharpening exponent

# evacuation route per position PAIR.  Pairs 1,2 (positions 2-5 = diag)
# must be 'd'.  Alternated so DVE (d: reduce_max) and ACT+GpSimd
# (gp: exp + 1-level ADD-tree sink, host sums the last 8 partials)
# evacuate concurrently; a gp pair leads so GpSimd starts early.
_PROUTE = ['gp', 'd', 'd', 'gp', 'd', 'gp', 'd', 'gp',
           'gp', 'd', 'gp', 'd', 'gp', 'd', 'gp', 'd']
_NGP = sum(r == 'gp' for r in _PROUTE)

_CACHE = {}


def _build_program():
    import concourse.bacc as bacc
    import concourse.mybir as mybir
    import concourse.tile as tile

    f32 = mybir.dt.float32
    bf16 = mybir.dt.bfloat16

    nc = bacc.Bacc("TRN2", num_devices=NC)
    inbuf = nc.declare_dram_parameter("inbuf", [128, MLOC + NT], bf16,
                                      isOutput=False)
    o_sim = nc.declare_dram_parameter("o_sim", [128, PT * BL], bf16,
                                      isOutput=True)
    o_gp = nc.declare_dram_parameter("o_gp", [128, _NGP * 1024], bf16,
                                     isOutput=True)

    X = mybir.AxisListType.X
    MAX = mybir.AluOpType.max
    ADD = mybir.AluOpType.add
    EXP = mybir.ActivationFunctionType.Exp

    with tile.TileContext(nc) as tc:
        with (
            tc.tile_pool(name="const", bufs=1) as cp,
            tc.tile_pool(name="sb", bufs=2) as sb,
            tc.tile_pool(name="eun", bufs=3) as ep,
            tc.tile_pool(name="tr", bufs=2) as tp,
            tc.tile_pool(name="pmm", bufs=3, space="PSUM") as pmm,
            tc.tile_pool(name="phb", bufs=1, space="PSUM") as phb,
        ):
            allin = cp.tile([128, MLOC + NT], bf16)
            im_T = allin[:, 0:MLOC]
            tn_T = allin[:, MLOC:MLOC + NT]
            sim_all = cp.tile([128, PT, BL], bf16)
            ones_bf = cp.tile([128, 1], bf16)
            nc.vector.memset(ones_bf[:], 1.0)

            with tc.high_priority():
                nc.sync.dma_start(allin[:, 0:MLOC + 512],
                                  inbuf[:, 0:MLOC + 512])
                nc.sync.dma_start(allin[:, MLOC + 512:MLOC + 2048],
                                  inbuf[:, MLOC + 512:MLOC + 2048])
            nc.sync.dma_start(allin[:, MLOC + 2048:MLOC + NT],
                              inbuf[:, MLOC + 2048:MLOC + NT])

            # preload the Exp table before the first route exp needs it
            dum = sb.tile([1, 1], f32, tag="dum", name="dum")
            nc.vector.memset(dum[:], 0.0)
            dum2 = sb.tile([1, 1], f32, tag="dum2", name="dum2")
            nc.scalar.activation(dum2[:], dum[:], EXP)

            # write-only PSUM bank for PE heartbeat matmuls: no reader, no
            # input deps, so one fires during any PE stall and keeps the
            # HAM activity window non-idle (PE stays at 2.4 GHz)
            hb = phb.tile([1, 64], f32, tag="hb", name="hb")

            cur_eun = None
            g_i = 0
            for pos in range(PT):
                pr, h = pos // 2, pos % 2
                r = _PROUTE[pr]
                nc.tensor.matmul(hb[:], lhsT=ones_bf[:],
                                 rhs=im_T[:, 0:64], start=True, stop=True,
                                 skip_group_check=True)
                ps = pmm.tile([128, 1024], f32, tag="ps", name=f"ps{pos}")
                for f in range(2):
                    nc.tensor.matmul(
                        ps[:, 512 * f:512 * (f + 1)],
                        lhsT=tn_T[:, 128 * pos:128 * (pos + 1)],
                        rhs=im_T[:, 512 * f:512 * (f + 1)],
                        start=True, stop=True,
                    )
                if r == 'd':
                    nc.vector.reduce_max(
                        sim_all[:, pos, :],
                        ps[:].rearrange("p (i j) -> p j i", j=BL),
                        axis=X,
                    )
                else:
                    if h == 0:
                        cur_eun = ep.tile([128, 2, 1024], bf16, tag="eun",
                                          name=f"eun{pr}")
                    nc.scalar.activation(cur_eun[:, h, :], ps[:], EXP,
                                         scale=SHARP)
                    if h == 1:
                        eun = cur_eun
                        t1 = tp.tile([128, 2, 512], bf16, tag="t1",
                                     name=f"t1_{pr}")
                        nc.gpsimd.tensor_tensor(t1[:], eun[:, :, 0:512],
                                                eun[:, :, 512:1024], op=ADD)
                        nc.sync.dma_start(
                            o_gp[:, 1024 * g_i:1024 * (g_i + 1)],
                            t1[:].rearrange("p q x -> p (q x)"))
                        g_i += 1
                if pos % 8 == 7:
                    g = pos // 8
                    nc.sync.dma_start(
                        o_sim[:, 512 * g:512 * (g + 1)],
                        sim_all[:, 8 * g:8 * (g + 1), :].rearrange(
                            "p q j -> p (q j)"))

    nc.finalize()
    return nc


DIAG_POS = 2  # own diag q's sit at positions DIAG_POS..DIAG_POS+4


def _perm(c):
    """q-block processing order for core c: own 4 diag q's at DIAG_POS."""
    own = list(range(4 * c, 4 * c + 4))
    rest = [q for q in range(PT) if q not in own]
    return rest[:DIAG_POS] + own + rest[DIAG_POS:]


def _get_program():
    if "nc" not in _CACHE:
        _CACHE["nc"] = _build_program()
    return _CACHE["nc"]


def _install_trace_shim():
    """Register the NTFF profile hook that this container's antenv lacks.

    Only used by the local test harness (KERNEL_TRACE=1); the grading
    path never enters here.
    """
    import sys
    import types
    import antenv
    import concourse.bass_utils as bu
    from trn_agent_boot.trn_boot import _ntff_profile_via_ctypes

    if "antenv.axon_hooks" not in sys.modules:
        hook = _ntff_profile_via_ctypes("/opt/axon/libaxon_pjrt.so")
        mod = types.ModuleType("antenv.axon_hooks")
        mod.get_axon_ntff_profile_hook = lambda: hook
        mod.set_axon_ntff_profile_hook = lambda h: None
        sys.modules["antenv.axon_hooks"] = mod
        antenv.axon_hooks = mod
    bu.upload_artifacts = lambda tmpdir: tmpdir


def kernel(img: np.ndarray, text: np.ndarray) -> np.ndarray:
    from concourse.bass_utils import run_bass_kernel_spmd

    nc = _get_program()
    img = np.asarray(img, dtype=np.float32)
    text = np.asarray(text, dtype=np.float32)

    # host: L2 normalize, d-major transpose, bf16
    tf = text.reshape(NT, D)
    tf = tf / np.maximum(np.sqrt((tf * tf).sum(-1, keepdims=True)), 1e-12)
    tn_full = np.ascontiguousarray(tf.T)

    imf = img.reshape(B * ZI, D)
    imf = imf / np.maximum(np.sqrt((imf * imf).sum(-1, keepdims=True)),
                           1e-12)
    imn = imf.reshape(B, ZI, D)

    blocks = tn_full.reshape(128, PT, 128)
    in_maps = []
    for c in range(NC):
        # img rows r = i*BL + j (i-major), transposed to [d, r]
        rows = imn[BL * c:BL * (c + 1)].transpose(1, 0, 2).reshape(MLOC, D)
        buf = np.empty((128, MLOC + NT), np.float32)
        buf[:, 0:MLOC] = rows.T
        buf[:, MLOC:] = blocks[:, _perm(c), :].reshape(128, NT)
        in_maps.append({"inbuf": buf.astype(ml_dtypes.bfloat16)})

    trace = bool(int(os.environ.get("KERNEL_TRACE", "0")))
    if trace:
        _install_trace_shim()
    r = run_bass_kernel_spmd(nc, in_maps, core_ids=list(range(NC)),
                             trace=trace)
    _CACHE["last_result"] = r

    # unshard + finish on host.  'd' position columns of o_sim hold raw
    # sims; 'gp' pairs emit 4 sharpened partial sums per (q, j) in o_gp.
    is_d = np.repeat(_PROUTE, 2) == 'd'
    gp_pairs = [pr for pr in range(NP) if _PROUTE[pr] == 'gp']
    total = 0.0
    den_t2i = np.zeros((128, PT), np.float64)
    pidx = np.arange(128)
    for c in range(NC):
        perm = np.array(_perm(c))
        v = np.asarray(r.results[c]["o_sim"], dtype=np.float64).reshape(
            128, PT, BL)
        e = np.where(is_d[None, :, None], np.exp(v), 0.0)
        gv = np.asarray(r.results[c]["o_gp"], dtype=np.float64).reshape(
            128, _NGP, 2, 8, BL)
        s = gv.sum(axis=3) ** (1.0 / SHARP)      # [128, _NGP, 2, BL]
        for g_i, pr in enumerate(gp_pairs):
            e[:, 2 * pr:2 * pr + 2, :] = s[:, g_i]
        den_t2i[:, perm] += e.sum(axis=2)
        total += float(np.sum(np.log(e.sum(axis=(0, 1)))))  # den_i2t local
        for k in range(4):
            total += DIAG_COEF * float(
                np.sum(v[pidx, DIAG_POS + k, 16 * k + pidx // 8]))
    total += float(np.sum(np.log(den_t2i)))
    return np.asarray(total, dtype=np.float32).reshape(())
